# revision 1
# baseline (speedup 1.0000x reference)
"""Trainium2 Bass kernel for CNNText: embedding gather + multi-width conv1d
+ bias/ReLU/max-pool + output matmul, data-parallel over batch on 8 NeuronCores.

Per core (8 batch elements):
  - Host: dedup words -> compact fp8(e4m3, x2^19) rows; ALL 8 batch elems'
    embeddings are host-pregathered into the DoubleRow pair layout (d-pairs
    per partition, K=256 per chunk), so the device needs no gather library,
    no Q7 ucode boot, and no gpsimd work at all.  Filters pre-transposed/
    scaled (x2^10) to fp8; scales fold back out in the ReLU's bias operand
    and the bf16 output layer (max-pool commutes with positive scaling).
  - Device: conv = PSUM-accumulated shifted matmuls (fp8 DoubleRow, ~216ns
    per K=512 x M=100 x N=512 pair, ~98% PE roofline); free-dim max reduce;
    relu(max+C*bias) -> bf16; [8,300]@[300,10] accumulated per width so the
    tail chain after the last conv matmul is one reduce+relu+matmul deep.
  - Startup: emb_b0 and the w3 filter tiles lead the two HWDGE queues so
    the first conv matmul can issue ~3us in; a short PE warmup burst keeps
    the HAM clock gate ramping while the first DMAs land.
"""
import os
# Fresh cores each init: hours of back-to-back NEFF runs leave the device
# in a drifted DVFS state worth ~+1-2us; a core reset at runtime init
# restores nominal clocks.  setdefault so an explicit harness choice wins.
os.environ.setdefault("NEURON_RT_RESET_CORES", "1")

import numpy as np
import ml_dtypes
from contextlib import ExitStack

import concourse.tile as tile
from concourse import bacc, mybir
from concourse.bass_utils import run_bass_kernel_spmd

# This image's antenv lacks axon_hooks; if tracing is requested via
# BASS_TRACE, bass_utils imports it. Provide a null shim so the run
# degrades to no-trace instead of crashing.
try:
    import antenv.axon_hooks  # noqa: F401
except ImportError:
    import sys as _sys
    import types as _types
    _m = _types.ModuleType("antenv.axon_hooks")
    _m.get_axon_ntff_profile_hook = lambda: None
    _m.set_axon_ntff_profile_hook = lambda h: None
    _sys.modules["antenv.axon_hooks"] = _m

P = 128
SL = 512
D = 512
B = 64
NCORES = 8
NB = B // NCORES
LAYERNUM = 100
WIDTHS = [3, 4, 5]
NT = sum(WIDTHS)          # 12 (width, offset) filter tiles
KC8 = 2                   # contraction chunks of 256 (d-pairs per partition)
NWARM = 45                # PE warmup matmuls while the first input DMAs land
NCOOL = 24                # post-conv dummy matmuls deferring the HAM drop
LPAD = 112                # filter dim padded so DR pair-dim strides are %16==0
DOUT = 10
S_E, S_K = 2.0**19, 2.0**10   # fp8 pre-scales for embedding / filters

F8 = mybir.dt.float8e4
F32 = mybir.dt.float32
BF16 = mybir.dt.bfloat16
NPF8 = ml_dtypes.float8_e4m3
NPBF16 = ml_dtypes.bfloat16

_CACHE: dict = {}
LAST_RESULTS = None


def _build():
    nc = bacc.Bacc("TRN2", target_bir_lowering=False, debug=False,
                   enable_asserts=True, num_devices=NCORES)

    emb_d = nc.dram_tensor("emb", [P, NB * KC8 * SL * 2], F8, kind="ExternalInput").ap()
    wts0_d = nc.dram_tensor("wts0", [P, KC8 * 2 * 1 * LPAD], F8, kind="ExternalInput").ap()
    wtsA_d = nc.dram_tensor("wtsA", [P, KC8 * 2 * 3 * LPAD], F8, kind="ExternalInput").ap()
    wtsB1_d = nc.dram_tensor("wtsB1", [P, KC8 * 2 * 4 * LAYERNUM], F8, kind="ExternalInput").ap()
    wtsB2_d = nc.dram_tensor("wtsB2", [P, KC8 * 2 * 4 * LAYERNUM], F8, kind="ExternalInput").ap()
    ol_d = nc.dram_tensor("ol", [LAYERNUM, 3 * DOUT], BF16, kind="ExternalInput").ap()
    bias_d = nc.dram_tensor("bias", [LAYERNUM, 3], F32, kind="ExternalInput").ap()
    out_d = nc.dram_tensor("out", [NB, DOUT], F32, kind="ExternalOutput").ap()
    scratch_d = nc.dram_tensor("scratch", [LAYERNUM, 1], F32, kind="ExternalOutput").ap()

    with tile.TileContext(nc) as tc:
        with ExitStack() as ctx:
            consts = ctx.enter_context(tc.tile_pool(name="consts", bufs=1))
            embp = ctx.enter_context(tc.tile_pool(name="emb", bufs=NB))
            psump = ctx.enter_context(tc.tile_pool(name="psum", bufs=2, space="PSUM"))
            outp = ctx.enter_context(tc.tile_pool(name="outp", bufs=1))

            emb_v = emb_d.rearrange("p (b j x) -> p b j x", b=NB, j=KC8)
            # b0's chunks are separate tiles so the first matmul is gated by
            # a 131KB transfer, not b0's full 262KB
            emb0j = [embp.tile([P, SL, 2], F8, tag=f"emb0j{j}", name=f"emb_b0j{j}")
                     for j in range(KC8)]
            embs = [None] + [embp.tile([P, KC8, SL, 2], F8, tag="emb",
                                       name=f"emb_b{b}") for b in range(1, NB)]
            # DR weight AP needs the pair-dim byte stride %16==0 (s3_lw.md,
            # checkMatmultPerfMode): LPAD=112 keeps 1/3/4 * 112 all %16==0
            wt0 = consts.tile([P, KC8, 2, 1, LPAD], F8)
            wtA = consts.tile([P, KC8, 2, 3, LPAD], F8)
            # 4-tile groups have e-stride 4*100=400 (%16==0) -> no pad
            wtB1 = consts.tile([P, KC8, 2, 4, LAYERNUM], F8)
            wtB2 = consts.tile([P, KC8, 2, 4, LAYERNUM], F8)

            # Both HWDGE queues are loaded in consumption order of the conv
            # stream: emb_b0/j0 + the t=0 filter tile gate the first matmul;
            # the rest of the weight set is split across the queues so it
            # lands before b0's w4/w5 matmuls need it.
            nc.sync.dma_start(
                emb0j[0][:].rearrange("p s e -> p (s e)"), emb_v[:, 0, 0])
            nc.scalar.dma_start(wt0[:], wts0_d.rearrange(
                "p (j e t f) -> p j e t f", j=KC8, e=2, t=1))
            nc.scalar.dma_start(wtA[:], wtsA_d.rearrange(
                "p (j e t f) -> p j e t f", j=KC8, e=2, t=3))
            nc.sync.dma_start(
                emb0j[1][:].rearrange("p s e -> p (s e)"), emb_v[:, 0, 1])
            nc.scalar.dma_start(wtB1[:], wtsB1_d.rearrange(
                "p (j e t f) -> p j e t f", j=KC8, e=2, t=4))
            nc.sync.dma_start(wtB2[:], wtsB2_d.rearrange(
                "p (j e t f) -> p j e t f", j=KC8, e=2, t=4))
            for b in range(1, NB):
                eng = nc.sync if b % 2 == 1 else nc.scalar
                eng.dma_start(
                    embs[b][:].rearrange("p j s e -> p (j s e)"),
                    emb_v[:, b].rearrange("p j x -> p (j x)"))
            ol_t = consts.tile([LAYERNUM, 3, DOUT], BF16)
            nc.scalar.dma_start(ol_t[:], ol_d.rearrange("p (w o) -> p w o", w=3))
            bias_t = consts.tile([LAYERNUM, 3], F32)
            nc.scalar.dma_start(bias_t[:], bias_d)

            pooled = [outp.tile([LAYERNUM, NB], F32, tag=f"pool{wi}", name=f"pool{wi}")
                      for wi in range(3)]

            # PE warmup: throwaway matmuls during the input-DMA wait keep the
            # HAM clock gate ramping toward 8/8 before the real stream starts.
            warm = consts.tile([P, P], F8, name="warm")
            nc.vector.memset(warm[:], 0)
            warm_ps = psump.tile([P, P], F32, tag="fin")
            for _ in range(NWARM):
                nc.tensor.matmul(warm_ps[:], lhsT=warm[:], rhs=warm[:],
                                 start=True, stop=True)

            for b in range(NB):
                t0 = 0
                for wi, w in enumerate(WIDTHS):
                    ps = psump.tile([LAYERNUM, SL], F32, tag=f"ps{wi}")
                    for i in range(w):
                        t = t0 + i
                        for j in range(KC8):
                            if t == 0:
                                lhsT = wt0[:, j, :, 0, 0:LAYERNUM]
                            elif t < 4:
                                lhsT = wtA[:, j, :, t - 1, 0:LAYERNUM]
                            elif t < 8:
                                lhsT = wtB1[:, j, :, t - 4, :]
                            else:
                                lhsT = wtB2[:, j, :, t - 8, :]
                            if b == 0:
                                rhs = emb0j[j][:, i:SL, :].rearrange(
                                    "p s e -> p e s")
                            else:
                                rhs = embs[b][:, j, i:SL, :].rearrange(
                                    "p s e -> p e s")
                            nc.tensor.matmul(
                                ps[:, 0:SL - i],
                                lhsT=lhsT,
                                rhs=rhs,
                                start=(i == 0 and j == 0),
                                stop=(i == w - 1 and j == KC8 - 1),
                                perf_mode=mybir.MatmulPerfMode.DoubleRow,
                            )
                    nc.vector.reduce_max(pooled[wi][:, b:b + 1], ps[:],
                                         axis=mybir.AxisListType.X)
                    t0 += w

            # Queue-warmer: a tiny DMA gated on b6's w5 pool write (~5us
            # before the end) keeps the sync DMA queue warm for the final out
            # transfer.  Exactly ONE warmer, and it must complete well before
            # the out: any additional DMA finishing in the kernel's last ~2us
            # reproducibly derails the teardown quiesce (+4..10us), whether
            # gated on b7's w4 or w5 pool writes.
            nc.sync.dma_start(scratch_d, pooled[2][:, 6:7], single_packet=True)

            fin = psump.tile([NB, DOUT], F32, tag="fin")
            for wi in range(3):
                pr = outp.tile([LAYERNUM, NB], BF16, tag=f"pr{wi}", name=f"pr{wi}")
                # relu((x + C*bias)) with C descaled via OL/C on host.  For
                # the last width (critical chain) split off b7's column so
                # only a [100,1] DVE op follows the final reduce; cols 0:7
                # are relu'd as soon as b6's reduce lands.
                if wi == 2:
                    nc.vector.tensor_scalar(pr[:, 0:NB - 1], pooled[wi][:, 0:NB - 1],
                                            scalar1=bias_t[:, wi:wi + 1], scalar2=0.0,
                                            op0=mybir.AluOpType.add,
                                            op1=mybir.AluOpType.max)
                    nc.vector.tensor_scalar(pr[:, NB - 1:NB], pooled[wi][:, NB - 1:NB],
                                            scalar1=bias_t[:, wi:wi + 1], scalar2=0.0,
                                            op0=mybir.AluOpType.add,
                                            op1=mybir.AluOpType.max)
                else:
                    nc.vector.tensor_scalar(pr[:], pooled[wi][:],
                                            scalar1=bias_t[:, wi:wi + 1], scalar2=0.0,
                                            op0=mybir.AluOpType.add,
                                            op1=mybir.AluOpType.max)
                nc.tensor.matmul(fin[:], lhsT=pr[:], rhs=ol_t[:, wi, :],
                                 start=(wi == 0), stop=(wi == 2))
            res = outp.tile([NB, DOUT], F32)
            nc.vector.tensor_copy(res[:], fin[:])
            nc.sync.dma_start(out_d, res[:], single_packet=True)
            # HAM throttles to 4/8 ~2.5us after the last real matmul — right
            # when the out transfer + teardown run.  A short dummy-matmul
            # burst (independent, queued behind the fin matmuls) defers the
            # drop past the quiesce so the tail runs at full clock.  It ends
            # before the teardown's tensor drain is reached, so it delays
            # nothing.
            if NCOOL:
                cool_ps = psump.tile([P, P], F32, tag="fin")
                for _ in range(NCOOL):
                    nc.tensor.matmul(cool_ps[:], lhsT=warm[:], rhs=warm[:],
                                     start=True, stop=True)

    nc.compile()
    return nc


def kernel(words, Embedding, outputlayer, filters_w3, bias_w3,
           filters_w4, bias_w4, filters_w5, bias_w5):
    global LAST_RESULTS
    words = np.asarray(words)
    Embedding = np.asarray(Embedding, dtype=np.float32)
    outputlayer = np.asarray(outputlayer, dtype=np.float32)
    filts = {3: np.asarray(filters_w3, dtype=np.float32),
             4: np.asarray(filters_w4, dtype=np.float32),
             5: np.asarray(filters_w5, dtype=np.float32)}
    biases = {3: np.asarray(bias_w3, dtype=np.float32),
              4: np.asarray(bias_w4, dtype=np.float32),
              5: np.asarray(bias_w5, dtype=np.float32)}

    # Dedup referenced vocab, cast only the used rows to scaled fp8, then
    # host-gather every batch element into the DoubleRow pair layout.
    uniq, inv = np.unique(words, return_inverse=True)
    table = (Embedding[uniq] * np.float32(S_E)).astype(NPF8)
    inv = inv.reshape(B, SL)

    K_all = np.stack([filts[w].reshape(LAYERNUM, w, D)[:, i, :].T
                      for w in WIDTHS for i in range(w)])    # [12, 512, 100]
    K8 = np.clip(K_all * np.float32(S_K), -240, 240).astype(NPF8)
    # lhsT pair layout: [p, j, e, t, m] padded m->LPAD; t split 1+3+8
    wts_full = np.zeros((P, KC8, 2, NT, LPAD), dtype=NPF8)
    wts_full[:, :, :, :, :LAYERNUM] = \
        K8.reshape(NT, KC8, P, 2, LAYERNUM).transpose(2, 1, 3, 0, 4)
    wts0 = wts_full[:, :, :, :1].reshape(P, KC8 * 2 * 1 * LPAD).copy()
    wtsA = wts_full[:, :, :, 1:4].reshape(P, KC8 * 2 * 3 * LPAD).copy()
    wtsB1 = wts_full[:, :, :, 4:8, :LAYERNUM] \
        .reshape(P, KC8 * 2 * 4 * LAYERNUM).copy()
    wtsB2 = wts_full[:, :, :, 8:, :LAYERNUM] \
        .reshape(P, KC8 * 2 * 4 * LAYERNUM).copy()
    C = np.float32(S_E * S_K)
    ol = (outputlayer.reshape(3, LAYERNUM, DOUT).transpose(1, 0, 2) / C) \
        .astype(NPBF16).reshape(LAYERNUM, 3 * DOUT).copy()
    bias = (np.stack([biases[w] for w in WIDTHS], axis=1) * C).copy()

    in_maps = []
    for core in range(NCORES):
        ridx = inv[core * NB:(core + 1) * NB]
        g = table[ridx]                                       # [NB, SL, D]
        e = (g.reshape(NB, SL, KC8, P, 2).transpose(3, 0, 2, 1, 4)
             .reshape(P, NB * KC8 * SL * 2).copy())
        in_maps.append({"emb": e, "wts0": wts0, "wtsA": wtsA,
                        "wtsB1": wtsB1, "wtsB2": wtsB2,
                        "ol": ol, "bias": bias})

    nc = _CACHE.get("nc")
    if nc is None:
        nc = _CACHE["nc"] = _build()

    res = run_bass_kernel_spmd(nc, in_maps, core_ids=list(range(NCORES)))
    LAST_RESULTS = res
    return np.concatenate([res.results[i]["out"] for i in range(NCORES)],
                          axis=0).astype(np.float32)



# revision 4
# speedup vs baseline: 1.1123x; 1.1123x over previous
"""Trainium2 Bass kernel for CNNText: embedding gather + multi-width conv1d
+ bias/ReLU/max-pool + output matmul, data-parallel over batch on 8 NeuronCores.

Per core (8 batch elements):
  - Host: dedup words -> compact fp8(e4m3, x2^19) rows; ALL 8 batch elems'
    embeddings are host-pregathered into the DoubleRow pair layout (d-pairs
    per partition, K=256 per chunk).  Filters pre-transposed/scaled (x2^10)
    to fp8, grouped t-major by conv width; scales fold back out in the
    ReLU's bias operand and the bf16 output layer (max-pool commutes with
    positive scaling).
  - Device: conv = PSUM-accumulated shifted matmuls (fp8 DoubleRow, width-
    OUTER loop so weight-group DMA deadlines trail the stream), free-dim
    max reduce, relu(max+C*bias) -> bf16, and a [10,NB]-oriented output
    matmul accumulated per width with only b7's w5 column on the tail
    chain (host transposes back).
  - Startup: the first conv matmul is gated by ONE "head" transfer
    (t0 weights + b0's full embedding) split across both HWDGE queues as
    partition halves; a PE warmup burst sized to the measured DMA landing
    (~10.9us: ~0.85us issue + transfer + ~1.7us completion receipt) keeps
    the HAM clock gate ramping so conv starts warm.
"""
import os
# Fresh cores each init: hours of back-to-back NEFF runs leave the device
# in a drifted DVFS state worth ~+1-2us; a core reset at runtime init
# restores nominal clocks.  setdefault so an explicit harness choice wins.
os.environ.setdefault("NEURON_RT_RESET_CORES", "1")

import numpy as np
import ml_dtypes
from contextlib import ExitStack

import concourse.tile as tile
from concourse import bacc, mybir
from concourse.bass_utils import run_bass_kernel_spmd

# This image's antenv lacks axon_hooks; if tracing is requested via
# BASS_TRACE, bass_utils imports it. Provide a null shim so the run
# degrades to no-trace instead of crashing.
try:
    import antenv.axon_hooks  # noqa: F401
except ImportError:
    import sys as _sys
    import types as _types
    _m = _types.ModuleType("antenv.axon_hooks")
    _m.get_axon_ntff_profile_hook = lambda: None
    _m.set_axon_ntff_profile_hook = lambda h: None
    _sys.modules["antenv.axon_hooks"] = _m

P = 128
SL = 512
D = 512
B = 64
NCORES = 8
NB = B // NCORES
LAYERNUM = 100
WIDTHS = [3, 4, 5]
NT = sum(WIDTHS)          # 12 (width, offset) filter tiles
KC8 = 2                   # contraction chunks of 256 (d-pairs per partition)
NWARM = 24                # PE warmup matmuls while the head DMA lands
NCOOL = 20                # post-conv dummy matmuls deferring the HAM drop
LPAD = 112                # filter dim padded so DR pair-dim strides are %16==0
DOUT = 10
S_E, S_K = 2.0**19, 2.0**10   # fp8 pre-scales for embedding / filters

F8 = mybir.dt.float8e4
F32 = mybir.dt.float32
BF16 = mybir.dt.bfloat16
NPF8 = ml_dtypes.float8_e4m3
NPBF16 = ml_dtypes.bfloat16

# tile t -> (group, local index); groups: head1=t0, head2=t1..2, wg4=t3..6,
# wg5=t7..11.  All groups use the t-major [p, t, j, e, f(LPAD)] layout so a
# (t, j) slice has pair-dim stride LPAD (%16==0, DR requirement).
_CACHE: dict = {}
LAST_RESULTS = None


def _build():
    nc = bacc.Bacc("TRN2", target_bir_lowering=False, debug=False,
                   enable_asserts=True, num_devices=NCORES)

    # head1: per partition [t0 weights (j,e,f=112 -> 448B) | b0 emb (2048B)]
    head1_d = nc.dram_tensor("head1", [P, 448 + KC8 * SL * 2], F8,
                             kind="ExternalInput").ap()
    head2_d = nc.dram_tensor("head2", [P, 2 * KC8 * 2 * LPAD], F8,
                             kind="ExternalInput").ap()
    wg4_d = nc.dram_tensor("wg4", [P, 4 * KC8 * 2 * LPAD], F8,
                           kind="ExternalInput").ap()
    wg5_d = nc.dram_tensor("wg5", [P, 5 * KC8 * 2 * LPAD], F8,
                           kind="ExternalInput").ap()
    embB_d = nc.dram_tensor("embB", [P, 7 * KC8 * SL * 2], F8,
                            kind="ExternalInput").ap()
    ol_d = nc.dram_tensor("ol", [LAYERNUM, 3 * DOUT], BF16, kind="ExternalInput").ap()
    bias_d = nc.dram_tensor("bias", [LAYERNUM, 3], F32, kind="ExternalInput").ap()
    out_d = nc.dram_tensor("out", [DOUT, NB], F32, kind="ExternalOutput").ap()
    scratch_d = nc.dram_tensor("scratch", [LAYERNUM, 1], F32, kind="ExternalOutput").ap()

    with tile.TileContext(nc) as tc:
        with ExitStack() as ctx:
            consts = ctx.enter_context(tc.tile_pool(name="consts", bufs=1))
            embp = ctx.enter_context(tc.tile_pool(name="emb", bufs=5))
            psump = ctx.enter_context(tc.tile_pool(name="psum", bufs=2, space="PSUM"))
            outp = ctx.enter_context(tc.tile_pool(name="outp", bufs=1))

            head1_t = consts.tile([P, 448 + KC8 * SL * 2], F8)
            head2_t = consts.tile([P, 2, KC8, 2, LPAD], F8)
            wg4_t = consts.tile([P, 4, KC8, 2, LPAD], F8)
            wg5_t = consts.tile([P, 5, KC8, 2, LPAD], F8)
            emb1_t = embp.tile([P, KC8, SL, 2], F8, tag="e1", name="emb_b1")
            emb2_t = embp.tile([P, KC8, SL, 2], F8, tag="e2", name="emb_b2")
            emb3_t = embp.tile([P, KC8, SL, 2], F8, tag="e3", name="emb_b3")
            emb45_t = embp.tile([P, 2, KC8, SL, 2], F8, tag="e45", name="emb_b45")
            emb67_t = embp.tile([P, 2, KC8, SL, 2], F8, tag="e67", name="emb_b67")

            wt0_v = head1_t[:, 0:448].rearrange("p (j e f) -> p j e f", j=KC8, e=2)
            emb0_v = head1_t[:, 448:448 + KC8 * SL * 2].rearrange(
                "p (j s x) -> p j s x", j=KC8, s=SL)
            embB_v = embB_d.rearrange("p (b j x) -> p b j x", b=7, j=KC8)

            # Both HWDGE queues in consumption order of the stream.  The
            # head (t0 weights + b0 emb) is split into partition halves so
            # both queues' SDMA port sets carry it; everything downstream
            # has >=0.3us of modeled slack against its first consumer.
            nc.sync.dma_start(head1_t[0:64, :], head1_d[0:64, :])
            nc.scalar.dma_start(head1_t[64:128, :], head1_d[64:128, :])
            nc.sync.dma_start(head2_t[:], head2_d.rearrange(
                "p (t j e f) -> p t j e f", t=2, j=KC8, e=2))
            nc.scalar.dma_start(
                emb1_t[:].rearrange("p j s x -> p (j s x)"),
                embB_v[:, 0].rearrange("p j x -> p (j x)"))
            nc.sync.dma_start(
                emb2_t[:].rearrange("p j s x -> p (j s x)"),
                embB_v[:, 1].rearrange("p j x -> p (j x)"))
            nc.scalar.dma_start(wg4_t[:], wg4_d.rearrange(
                "p (t j e f) -> p t j e f", t=4, j=KC8, e=2))
            nc.scalar.dma_start(
                emb3_t[:].rearrange("p j s x -> p (j s x)"),
                embB_v[:, 2].rearrange("p j x -> p (j x)"))
            nc.sync.dma_start(
                emb45_t[:].rearrange("p b j s x -> p (b j s x)"),
                embB_v[:, 3:5].rearrange("p b j x -> p (b j x)"))
            nc.scalar.dma_start(wg5_t[:], wg5_d.rearrange(
                "p (t j e f) -> p t j e f", t=5, j=KC8, e=2))
            nc.scalar.dma_start(
                emb67_t[:].rearrange("p b j s x -> p (b j s x)"),
                embB_v[:, 5:7].rearrange("p b j x -> p (b j x)"))
            ol_t = consts.tile([LAYERNUM, 3, DOUT], BF16)
            nc.scalar.dma_start(ol_t[:], ol_d.rearrange("p (w o) -> p w o", w=3))
            bias_t = consts.tile([LAYERNUM, 3], F32)
            nc.scalar.dma_start(bias_t[:], bias_d)

            def lhsT_for(t, j):
                if t == 0:
                    return wt0_v[:, j, :, 0:LAYERNUM]
                if t < 3:
                    return head2_t[:, t - 1, j, :, 0:LAYERNUM]
                if t < 7:
                    return wg4_t[:, t - 3, j, :, 0:LAYERNUM]
                return wg5_t[:, t - 7, j, :, 0:LAYERNUM]

            def rhs_for(b, j, i):
                if b == 0:
                    return emb0_v[:, j, i:SL, :].rearrange("p s e -> p e s")
                if b == 1:
                    return emb1_t[:, j, i:SL, :].rearrange("p s e -> p e s")
                if b == 2:
                    return emb2_t[:, j, i:SL, :].rearrange("p s e -> p e s")
                if b == 3:
                    return emb3_t[:, j, i:SL, :].rearrange("p s e -> p e s")
                if b < 6:
                    return emb45_t[:, b - 4, j, i:SL, :].rearrange("p s e -> p e s")
                return emb67_t[:, b - 6, j, i:SL, :].rearrange("p s e -> p e s")

            pooled = [outp.tile([LAYERNUM, NB], F32, tag=f"pool{wi}", name=f"pool{wi}")
                      for wi in range(3)]
            prs = [None, None, None]

            # PE warmup: throwaway matmuls during the head-DMA wait keep the
            # HAM clock gate ramping toward 8/8 before the real stream
            # starts.  GpSimd does the memset (it exits the start handshake
            # first), so warmup begins ~0.5us earlier than a DVE memset.
            warm = consts.tile([P, P], F8, name="warm")
            nc.gpsimd.memset(warm[:], 0)
            warm_ps = psump.tile([P, P], F32, tag="warm")
            for _ in range(NWARM):
                nc.tensor.matmul(warm_ps[:], lhsT=warm[:], rhs=warm[:],
                                 start=True, stop=True)

            fin2 = psump.tile([DOUT, NB], F32, tag="fin")

            def relu(wi, c0, c1):
                pr = prs[wi]
                nc.vector.tensor_scalar(pr[:, c0:c1], pooled[wi][:, c0:c1],
                                        scalar1=bias_t[:, wi:wi + 1], scalar2=0.0,
                                        op0=mybir.AluOpType.add,
                                        op1=mybir.AluOpType.max)

            t0s = [0, 3, 7]
            for wi, w in enumerate(WIDTHS):
                prs[wi] = outp.tile([LAYERNUM, NB], BF16, tag=f"pr{wi}",
                                    name=f"pr{wi}")
                for b in range(NB):
                    ps = psump.tile([LAYERNUM, SL], F32, tag=f"ps{b % 2}")
                    nmm = 0
                    for i in range(w):
                        t = t0s[wi] + i
                        for j in range(KC8):
                            nc.tensor.matmul(
                                ps[:, 0:SL - i],
                                lhsT=lhsT_for(t, j),
                                rhs=rhs_for(b, j, i),
                                start=(i == 0 and j == 0),
                                stop=(i == w - 1 and j == KC8 - 1),
                                perf_mode=mybir.MatmulPerfMode.DoubleRow,
                            )
                            nmm += 1
                            # For w5/b7 slip the cols-0:6 output matmul into
                            # the conv stream once b6's relu has had time to
                            # land (~6 MMs after b6's last conv matmul).
                            if wi == 2 and b == 7 and nmm == 6:
                                nc.tensor.matmul(fin2[:, 0:NB - 1],
                                                 lhsT=ol_t[:, 2, :],
                                                 rhs=prs[2][:, 0:NB - 1],
                                                 start=False, stop=False)
                    nc.vector.reduce_max(pooled[wi][:, b:b + 1], ps[:],
                                         axis=mybir.AxisListType.X)
                    # Deferred cross-width work, placed so the PE queue
                    # never stalls on a DVE dependency:
                    if wi == 1 and b == 2:
                        relu(0, 0, NB)
                        nc.tensor.matmul(fin2[:], lhsT=ol_t[:, 0, :],
                                         rhs=prs[0][:], start=True, stop=False)
                    if wi == 2 and b == 2:
                        relu(1, 0, NB)
                        nc.tensor.matmul(fin2[:], lhsT=ol_t[:, 1, :],
                                         rhs=prs[1][:], start=False, stop=False)
                    if wi == 2 and b == 6:
                        relu(2, 0, NB - 1)

            # Queue-warmer: a tiny DMA gated on b6's w5 pool write (~3us
            # before the end) keeps the sync DMA queue warm for the final
            # out transfer.
            nc.sync.dma_start(scratch_d, pooled[2][:, 6:7], single_packet=True)

            # Tail: only b7's w5 column chains after the final reduce.
            relu(2, NB - 1, NB)
            nc.tensor.matmul(fin2[:, NB - 1:NB], lhsT=ol_t[:, 2, :],
                             rhs=prs[2][:, NB - 1:NB], start=False, stop=True)
            res = outp.tile([DOUT, NB], F32)
            nc.scalar.copy(res[:], fin2[:])
            nc.sync.dma_start(out_d, res[:], single_packet=True)
            # HAM throttles to 4/8 ~2.5us after the last real matmul — right
            # when the out transfer + teardown run.  A short dummy-matmul
            # burst defers the drop past the quiesce so the tail runs at
            # full clock.
            if NCOOL:
                cool_ps = psump.tile([P, P], F32, tag="warm")
                for _ in range(NCOOL):
                    nc.tensor.matmul(cool_ps[:], lhsT=warm[:], rhs=warm[:],
                                     start=True, stop=True)

    nc.compile()
    return nc


def kernel(words, Embedding, outputlayer, filters_w3, bias_w3,
           filters_w4, bias_w4, filters_w5, bias_w5):
    global LAST_RESULTS
    words = np.asarray(words)
    Embedding = np.asarray(Embedding, dtype=np.float32)
    outputlayer = np.asarray(outputlayer, dtype=np.float32)
    filts = {3: np.asarray(filters_w3, dtype=np.float32),
             4: np.asarray(filters_w4, dtype=np.float32),
             5: np.asarray(filters_w5, dtype=np.float32)}
    biases = {3: np.asarray(bias_w3, dtype=np.float32),
              4: np.asarray(bias_w4, dtype=np.float32),
              5: np.asarray(bias_w5, dtype=np.float32)}

    # Dedup referenced vocab, cast only the used rows to scaled fp8, then
    # host-gather every batch element into the DoubleRow pair layout.
    uniq, inv = np.unique(words, return_inverse=True)
    table = (Embedding[uniq] * np.float32(S_E)).astype(NPF8)
    inv = inv.reshape(B, SL)

    K_all = np.stack([filts[w].reshape(LAYERNUM, w, D)[:, i, :].T
                      for w in WIDTHS for i in range(w)])    # [12, 512, 100]
    K8 = np.clip(K_all * np.float32(S_K), -240, 240).astype(NPF8)
    # DR pair layout per tile: [j, p, e, m]; groups are t-major [p, t, j, e, f]
    K8r = K8.reshape(NT, KC8, P, 2, LAYERNUM)

    def group(ts):
        g = np.zeros((P, len(ts), KC8, 2, LPAD), dtype=NPF8)
        for tl, t in enumerate(ts):
            g[:, tl, :, :, :LAYERNUM] = K8r[t].transpose(1, 0, 2, 3)
        return g

    h1w = group([0]).reshape(P, KC8 * 2 * LPAD)              # [P, 448]
    head2 = group([1, 2]).reshape(P, 2 * KC8 * 2 * LPAD).copy()
    wg4 = group([3, 4, 5, 6]).reshape(P, 4 * KC8 * 2 * LPAD).copy()
    wg5 = group([7, 8, 9, 10, 11]).reshape(P, 5 * KC8 * 2 * LPAD).copy()

    C = np.float32(S_E * S_K)
    ol = (outputlayer.reshape(3, LAYERNUM, DOUT).transpose(1, 0, 2) / C) \
        .astype(NPBF16).reshape(LAYERNUM, 3 * DOUT).copy()
    bias = (np.stack([biases[w] for w in WIDTHS], axis=1) * C).copy()

    in_maps = []
    for core in range(NCORES):
        ridx = inv[core * NB:(core + 1) * NB]
        g = table[ridx]                                       # [NB, SL, D]
        e = (g.reshape(NB, SL, KC8, P, 2).transpose(3, 0, 2, 1, 4)
             .reshape(P, NB, KC8 * SL * 2))
        head1 = np.concatenate([h1w, e[:, 0]], axis=1).copy()
        embB = e[:, 1:].reshape(P, 7 * KC8 * SL * 2).copy()
        in_maps.append({"head1": head1, "head2": head2, "wg4": wg4,
                        "wg5": wg5, "embB": embB, "ol": ol, "bias": bias})

    nc = _CACHE.get("nc")
    if nc is None:
        nc = _CACHE["nc"] = _build()

    res = run_bass_kernel_spmd(nc, in_maps, core_ids=list(range(NCORES)))
    LAST_RESULTS = res
    return np.concatenate([res.results[i]["out"].T for i in range(NCORES)],
                          axis=0).astype(np.float32)


# revision 11
# speedup vs baseline: 1.1351x; 1.0205x over previous
"""Trainium2 Bass kernel for CNNText: embedding gather + multi-width conv1d
+ bias/ReLU/max-pool + output matmul, data-parallel over batch on 8 NeuronCores.

Per core (8 batch elements):
  - Host: dedup words -> compact fp8(e4m3, x2^19) rows; ALL 8 batch elems'
    embeddings are host-pregathered into the DoubleRow pair layout (d-pairs
    per partition, K=256 per chunk).  Filters pre-transposed/scaled (x2^10)
    to fp8, grouped t-major by conv width; scales fold back out in the
    ReLU's bias operand and the bf16 output layer (max-pool commutes with
    positive scaling).
  - Device: conv = PSUM-accumulated shifted matmuls (fp8 DoubleRow, width-
    OUTER loop so weight-group DMA deadlines trail the stream), free-dim
    max reduce, relu(max+C*bias) -> bf16, and a [10,NB]-oriented output
    matmul accumulated per width with only b7's w5 column on the tail
    chain (host transposes back).
  - Startup: the first conv matmul is gated by ONE "head" transfer
    (t0 weights + b0's full embedding) split across both HWDGE queues as
    partition halves; a PE warmup burst sized to the measured DMA landing
    (~10.9us: ~0.85us issue + transfer + ~1.7us completion receipt) keeps
    the HAM clock gate ramping so conv starts warm.
"""
import os
# Fresh cores each init: hours of back-to-back NEFF runs leave the device
# in a drifted DVFS state worth ~+1-2us; a core reset at runtime init
# restores nominal clocks.  setdefault so an explicit harness choice wins.
os.environ.setdefault("NEURON_RT_RESET_CORES", "1")

import numpy as np
import ml_dtypes
from contextlib import ExitStack

import concourse.tile as tile
from concourse import bacc, mybir
from concourse.bass_utils import run_bass_kernel_spmd

# This image's antenv lacks axon_hooks; if tracing is requested via
# BASS_TRACE, bass_utils imports it. Provide a null shim so the run
# degrades to no-trace instead of crashing.
try:
    import antenv.axon_hooks  # noqa: F401
except ImportError:
    import sys as _sys
    import types as _types
    _m = _types.ModuleType("antenv.axon_hooks")
    _m.get_axon_ntff_profile_hook = lambda: None
    _m.set_axon_ntff_profile_hook = lambda h: None
    _sys.modules["antenv.axon_hooks"] = _m

P = 128
SL = 512
D = 512
B = 64
NCORES = 8
NB = B // NCORES
LAYERNUM = 100
WIDTHS = [3, 4, 5]
NT = sum(WIDTHS)          # 12 (width, offset) filter tiles
KC8 = 2                   # contraction chunks of 256 (d-pairs per partition)
NWARM = 40                # PE warmup matmuls while the head DMA lands
NCOOL = 20                # post-conv dummy matmuls deferring the HAM drop
LPAD = 112                # filter dim padded so DR pair-dim strides are %16==0
DOUT = 10
S_E, S_K = 2.0**19, 2.0**10   # fp8 pre-scales for embedding / filters

F8 = mybir.dt.float8e4
F32 = mybir.dt.float32
BF16 = mybir.dt.bfloat16
NPF8 = ml_dtypes.float8_e4m3
NPBF16 = ml_dtypes.bfloat16

# tile t -> (group, local index); groups: head1=t0, head2=t1..2, wg4=t3..6,
# wg5=t7..11.  All groups use the t-major [p, t, j, e, f(LPAD)] layout so a
# (t, j) slice has pair-dim stride LPAD (%16==0, DR requirement).
_CACHE: dict = {}
LAST_RESULTS = None


def _build():
    nc = bacc.Bacc("TRN2", target_bir_lowering=False, debug=False,
                   enable_asserts=True, num_devices=NCORES)

    # head: per partition [w3 weights t0..t2 (t,j,e,f=112 -> 1344B) |
    # b0 emb (2048B) | b1 emb (2048B)] — everything the first ~1.3us of the
    # conv stream needs, landed as ONE split-halved transfer per queue.
    HW3 = 3 * KC8 * 2 * LPAD                     # 1344
    head_d = nc.dram_tensor("head", [P, HW3 + 2 * KC8 * SL * 2], F8,
                            kind="ExternalInput").ap()
    # w4+w5 weights combined (consumed from T0+10us on)
    wg45_d = nc.dram_tensor("wg45", [P, 9 * KC8 * 2 * LPAD], F8,
                            kind="ExternalInput").ap()
    embB_d = nc.dram_tensor("embB", [P, 6 * KC8 * SL * 2], F8,
                            kind="ExternalInput").ap()
    ol_d = nc.dram_tensor("ol", [LAYERNUM, 3 * DOUT], BF16, kind="ExternalInput").ap()
    bias_d = nc.dram_tensor("bias", [LAYERNUM, 3], F32, kind="ExternalInput").ap()
    out_d = nc.dram_tensor("out", [DOUT, NB], F32, kind="ExternalOutput").ap()
    scratch_d = nc.dram_tensor("scratch", [LAYERNUM, 1], F32, kind="ExternalOutput").ap()

    with tile.TileContext(nc) as tc:
        with ExitStack() as ctx:
            consts = ctx.enter_context(tc.tile_pool(name="consts", bufs=1))
            embp = ctx.enter_context(tc.tile_pool(name="emb", bufs=1))
            psump = ctx.enter_context(tc.tile_pool(name="psum", bufs=2, space="PSUM"))
            outp = ctx.enter_context(tc.tile_pool(name="outp", bufs=1))

            head_t = consts.tile([P, HW3 + 2 * KC8 * SL * 2], F8)
            wg45_t = consts.tile([P, 9, KC8, 2, LPAD], F8)
            embs = [embp.tile([P, KC8, SL, 2], F8, tag=f"e{b}", name=f"emb_b{b}")
                    for b in range(2, 8)]

            wt3_v = head_t[:, 0:HW3].rearrange(
                "p (t j e f) -> p t j e f", t=3, j=KC8, e=2)
            emb01_v = head_t[:, HW3:HW3 + 2 * KC8 * SL * 2].rearrange(
                "p (b j s x) -> p b j s x", b=2, j=KC8, s=SL)
            embB_v = embB_d.rearrange("p (b j x) -> p b j x", b=6, j=KC8)

            # Both HWDGE queues in consumption order of the stream.  DMA
            # completion receipts (~1-1.9us) serialize per queue, so the
            # whole first ~1.3us of conv consumption rides in ONE head
            # transfer (halved across the queues); every later transfer has
            # >=1.3us of modeled slack against its first consumer.
            nc.sync.dma_start(head_t[0:64, :], head_d[0:64, :])
            nc.scalar.dma_start(head_t[64:128, :], head_d[64:128, :])
            for k, b in enumerate(range(2, 8)):
                eng = nc.sync if b % 2 == 0 else nc.scalar
                eng.dma_start(
                    embs[k][:].rearrange("p j s x -> p (j s x)"),
                    embB_v[:, k].rearrange("p j x -> p (j x)"))
            nc.scalar.dma_start(wg45_t[:], wg45_d.rearrange(
                "p (t j e f) -> p t j e f", t=9, j=KC8, e=2))
            ol_t = consts.tile([LAYERNUM, 3, DOUT], BF16)
            nc.scalar.dma_start(ol_t[:], ol_d.rearrange("p (w o) -> p w o", w=3))
            bias_t = consts.tile([LAYERNUM, 3], F32)
            nc.scalar.dma_start(bias_t[:], bias_d)

            def lhsT_for(t, j):
                if t < 3:
                    return wt3_v[:, t, j, :, 0:LAYERNUM]
                return wg45_t[:, t - 3, j, :, 0:LAYERNUM]

            def rhs_for(b, j, i):
                if b < 2:
                    return emb01_v[:, b, j, i:SL, :].rearrange("p s e -> p e s")
                return embs[b - 2][:, j, i:SL, :].rearrange("p s e -> p e s")

            pooled = [outp.tile([LAYERNUM, NB], F32, tag=f"pool{wi}", name=f"pool{wi}")
                      for wi in range(3)]
            prs = [None, None, None]

            # PE warmup: throwaway matmuls during the head-DMA wait keep the
            # HAM clock gate ramping toward 8/8 before the real stream
            # starts.  GpSimd does the memset (it exits the start handshake
            # first), so warmup begins ~0.5us earlier than a DVE memset.
            warm = consts.tile([P, P], F8, name="warm")
            nc.gpsimd.memset(warm[:], 0)
            warm_ps = psump.tile([P, P], F32, tag="warm")
            for _ in range(NWARM):
                nc.tensor.matmul(warm_ps[:], lhsT=warm[:], rhs=warm[:],
                                 start=True, stop=True)

            fin2 = psump.tile([DOUT, NB], F32, tag="fin")

            def relu(wi, c0, c1):
                pr = prs[wi]
                nc.vector.tensor_scalar(pr[:, c0:c1], pooled[wi][:, c0:c1],
                                        scalar1=bias_t[:, wi:wi + 1], scalar2=0.0,
                                        op0=mybir.AluOpType.add,
                                        op1=mybir.AluOpType.max)

            t0s = [0, 3, 7]
            for wi, w in enumerate(WIDTHS):
                prs[wi] = outp.tile([LAYERNUM, NB], BF16, tag=f"pr{wi}",
                                    name=f"pr{wi}")
                for b in range(NB):
                    ps = psump.tile([LAYERNUM, SL], F32, tag=f"ps{b % 2}")
                    for i in range(w):
                        t = t0s[wi] + i
                        for j in range(KC8):
                            nc.tensor.matmul(
                                ps[:, 0:SL - i],
                                lhsT=lhsT_for(t, j),
                                rhs=rhs_for(b, j, i),
                                start=(i == 0 and j == 0),
                                stop=(i == w - 1 and j == KC8 - 1),
                                perf_mode=mybir.MatmulPerfMode.DoubleRow,
                            )
                    nc.vector.reduce_max(pooled[wi][:, b:b + 1], ps[:],
                                         axis=mybir.AxisListType.X)
                    # Deferred relus (DVE-only; the fp8->bf16 fin matmuls
                    # stay OUT of the conv stream — each DR<->normal PE
                    # mode switch costs ~0.4us of pipeline flush):
                    if wi == 1 and b == 2:
                        relu(0, 0, NB)
                    if wi == 2 and b == 2:
                        relu(1, 0, NB)
                    if wi == 2 and b == 6:
                        relu(2, 0, NB - 1)

            # Queue-warmer: a tiny DMA gated on b6's w5 pool write (~2-3us
            # before the end) keeps the sync DMA queue warm for the final
            # out transfer.
            nc.sync.dma_start(scratch_d, pooled[2][:, 6:7], single_packet=True)

            # Tail: the first three fin matmuls depend only on already-
            # relu'd columns, so the PE runs them (one mode switch) while
            # the DVE does b7's w5 reduce; only the single-column w5b
            # matmul chains after it.
            nc.tensor.matmul(fin2[:], lhsT=ol_t[:, 0, :],
                             rhs=prs[0][:], start=True, stop=False)
            nc.tensor.matmul(fin2[:], lhsT=ol_t[:, 1, :],
                             rhs=prs[1][:], start=False, stop=False)
            nc.tensor.matmul(fin2[:, 0:NB - 1], lhsT=ol_t[:, 2, :],
                             rhs=prs[2][:, 0:NB - 1], start=False, stop=False)
            relu(2, NB - 1, NB)
            nc.tensor.matmul(fin2[:, NB - 1:NB], lhsT=ol_t[:, 2, :],
                             rhs=prs[2][:, NB - 1:NB], start=False, stop=True)
            res = outp.tile([DOUT, NB], F32)
            nc.scalar.copy(res[:], fin2[:])
            nc.sync.dma_start(out_d, res[:], single_packet=True)
            # HAM throttles to 4/8 ~2.5us after the last real matmul — right
            # when the out transfer + teardown run.  A short dummy-matmul
            # burst defers the drop past the quiesce so the tail runs at
            # full clock.
            if NCOOL:
                cool_ps = psump.tile([P, P], F32, tag="warm")
                for _ in range(NCOOL):
                    nc.tensor.matmul(cool_ps[:], lhsT=warm[:], rhs=warm[:],
                                     start=True, stop=True)

    nc.compile()
    return nc


def kernel(words, Embedding, outputlayer, filters_w3, bias_w3,
           filters_w4, bias_w4, filters_w5, bias_w5):
    global LAST_RESULTS
    words = np.asarray(words)
    Embedding = np.asarray(Embedding, dtype=np.float32)
    outputlayer = np.asarray(outputlayer, dtype=np.float32)
    filts = {3: np.asarray(filters_w3, dtype=np.float32),
             4: np.asarray(filters_w4, dtype=np.float32),
             5: np.asarray(filters_w5, dtype=np.float32)}
    biases = {3: np.asarray(bias_w3, dtype=np.float32),
              4: np.asarray(bias_w4, dtype=np.float32),
              5: np.asarray(bias_w5, dtype=np.float32)}

    # Dedup referenced vocab, cast only the used rows to scaled fp8, then
    # host-gather every batch element into the DoubleRow pair layout.
    uniq, inv = np.unique(words, return_inverse=True)
    table = (Embedding[uniq] * np.float32(S_E)).astype(NPF8)
    inv = inv.reshape(B, SL)

    K_all = np.stack([filts[w].reshape(LAYERNUM, w, D)[:, i, :].T
                      for w in WIDTHS for i in range(w)])    # [12, 512, 100]
    K8 = np.clip(K_all * np.float32(S_K), -240, 240).astype(NPF8)
    # DR pair layout per tile: [j, p, e, m]; groups are t-major [p, t, j, e, f]
    K8r = K8.reshape(NT, KC8, P, 2, LAYERNUM)

    def group(ts):
        g = np.zeros((P, len(ts), KC8, 2, LPAD), dtype=NPF8)
        for tl, t in enumerate(ts):
            g[:, tl, :, :, :LAYERNUM] = K8r[t].transpose(1, 0, 2, 3)
        return g

    hw3 = group([0, 1, 2]).reshape(P, 3 * KC8 * 2 * LPAD)    # [P, 1344]
    wg45 = group(list(range(3, 12))).reshape(P, 9 * KC8 * 2 * LPAD).copy()

    C = np.float32(S_E * S_K)
    ol = (outputlayer.reshape(3, LAYERNUM, DOUT).transpose(1, 0, 2) / C) \
        .astype(NPBF16).reshape(LAYERNUM, 3 * DOUT).copy()
    bias = (np.stack([biases[w] for w in WIDTHS], axis=1) * C).copy()

    in_maps = []
    for core in range(NCORES):
        ridx = inv[core * NB:(core + 1) * NB]
        g = table[ridx]                                       # [NB, SL, D]
        e = (g.reshape(NB, SL, KC8, P, 2).transpose(3, 0, 2, 1, 4)
             .reshape(P, NB, KC8 * SL * 2))
        head = np.concatenate([hw3, e[:, 0], e[:, 1]], axis=1).copy()
        embB = e[:, 2:].reshape(P, 6 * KC8 * SL * 2).copy()
        in_maps.append({"head": head, "wg45": wg45,
                        "embB": embB, "ol": ol, "bias": bias})

    nc = _CACHE.get("nc")
    if nc is None:
        nc = _CACHE["nc"] = _build()

    res = run_bass_kernel_spmd(nc, in_maps, core_ids=list(range(NCORES)))
    LAST_RESULTS = res
    return np.concatenate([res.results[i]["out"].T for i in range(NCORES)],
                          axis=0).astype(np.float32)
